# revision 49
# baseline (speedup 1.0000x reference)
"""Trainium2 Bass kernel for EnhancedFastKANLayer.

Reference computation (B=16384, D=O=512, G=8 grids):
    x_norm = (x - mean) * rsqrt(var + eps) * gamma + beta          # BN inference
    basis[b,d,g] = exp(-((x_norm[b,d] - grid[g]) / denom)^2)       # RBF expansion
    out = basis.reshape(B, D*G) @ W_spline + b_spline
        + relu(x) @ W_base + b_base + x

Strategy:
  - Data parallel: batch 16384 sharded 8 ways (2048 rows/core); weights
    replicated. No collectives.
  - All on-chip compute happens in the transposed layout [feature, batch]:
    the output is produced as out_T [O, B_shard] and transposed back on the
    host. This makes BN/basis per-partition-scalar ops and lets the spline
    matmul consume basis tiles directly as the moving operand.
  - RBF via ScalarE Derivative_Erf: d/dx erf(x) = 2/sqrt(pi)*exp(-x^2), so
    basis_g = sqrt(pi)/2 * Derivative_Erf(uscale*x + (ushift - c_g)) -- ONE
    ACT op per (d-tile, grid): BN is folded into the ACT per-partition
    scale+bias operands, and sqrt(pi)/2 into W_spline on the host.
  - DMA cost on this part is ~0.4us per partition-row packet per ring,
    nearly independent of row size, so the kernel minimizes DMA count and
    maximizes row length:
      * x is pre-transposed/pre-cast to fp16 [D, B_shard] and loaded in 4
        DMAs of [128, 2128] (4.25KB rows) covering BOTH batch chunks; the
        first 80 fp16 columns of the d-tile-0 rows carry all fp32 scalar
        params bit-cast into fp16 pairs (recovered on chip via bitcast).
      * the first spline-weight slice rides the otherwise-idle Activation
        HWDGE queue so it lands in parallel with x/params on the sync queue.
      * remaining weights stream on the gpsimd SWDGE queue in 7KB rows.
  - Matmul: out_T[o_sub, b] accumulates 36 matmuls per PSUM tile: 32 spline
    K-chunks + 4 relu(x)@W_base K-chunks.  The residual (+x) and the output
    bias are fused into the single epilogue DVE op per psum tile:
        ot = (psum + bias) + x_T        (scalar_tensor_tensor)
  - Matmul emission is kc-outer (all 8 PSUM tiles per K-chunk before the
    next K-chunk): the PE consumes one basis tile per 1.73us while ACT
    produces one per ~1.4us, so the pipeline runs stall-free from the first
    tile with no extra lookahead needed.
  - Output is fp16 (upcast on host), one DMA per 128-row osub across the
    whole 1024-col chunk (2KB rows); the last chunk's stores are split
    across the sync and Activation HWDGE queues to halve the final drain.
  - PE warm-up matmuls on an uninitialized SBUF region (results discarded
    by the first real matmul's start=True bank reset) release the HAM clock
    throttle (~3.4us of sustained PE activity at half clock) while the
    ramp-critical DMAs land; the warm-up count is tuned so the PE rolls
    straight from warm-up into the real stream with no idle gap (an idle
    PE re-arms the throttle).
"""

import numpy as np
from contextlib import ExitStack

import concourse.bass as bass
import concourse.tile as tile
from concourse import bacc, mybir
from concourse._compat import with_exitstack
from concourse.bass_utils import run_bass_kernel_spmd

N_CORES = 8
BATCH, IN_DIM, OUT_DIM, G = 16384, 512, 512, 8
B_SHARD = BATCH // N_CORES          # 2048
B_CHUNK = 1024                      # batch columns processed per chunk
GRID_MIN, GRID_MAX, BN_EPS = -2.0, 2.0, 1e-3
DENOM = (GRID_MAX - GRID_MIN) / G   # 0.5
N_DT = IN_DIM // 128                # 4 d-tiles
K_SPLINE = N_DT * G                 # 32 spline K-chunks
K_BASE = N_DT                       # 4 base K-chunks
N_OSUB = OUT_DIM // 128             # 4 output partition tiles
N_PAR = N_DT + N_DT * G + N_OSUB    # 40 fp32 scalar params
P_PRE = 2 * N_PAR                   # 80 fp16 columns carrying the params

F32 = mybir.dt.float32
F16 = mybir.dt.float16

N_WARM = 74                         # PE warm-up matmuls (HAM clock ramp)


def _grid_consts():
    grid = np.linspace(GRID_MIN, GRID_MAX, G, dtype=np.float32)
    c = (grid / np.float32(DENOM)).astype(np.float32)        # grid in u-units
    return c


@with_exitstack
def _body(ctx, tc, xaug, w_sp, w_b, out_t, b_shard, b_chunk):
    nc = tc.nc
    n_chunks = b_shard // b_chunk
    n_bh = b_chunk // 512            # 512-wide moving-operand slices
    k_total = K_SPLINE + K_BASE      # residual handled in the epilogue
    KB = 6                           # K-chunks per PE block
    K_LAST = 30                      # kc >= K_LAST run per-bank, fused w/ epilogue
    W_SLICE = 7                      # spline weight K-chunks per SWDGE DMA

    const_pool = ctx.enter_context(tc.tile_pool(name="const", bufs=1))
    w_pool = ctx.enter_context(tc.tile_pool(name="w", bufs=1))
    x_pool = ctx.enter_context(tc.tile_pool(name="x", bufs=1))
    # spline basis tiles: 32 per chunk stay resident through the chunk's
    # matmul phase; extra slots let the next chunk's production run ahead.
    basis_pool = ctx.enter_context(tc.tile_pool(name="basis", bufs=K_SPLINE + 12))
    psum_pool = ctx.enter_context(
        tc.tile_pool(name="psum", bufs=8, space="PSUM"))
    out_pool = ctx.enter_context(tc.tile_pool(name="outs", bufs=6))

    # ---- warm-up operand: a raw (untracked) SBUF region read
    # uninitialized, so the PE warm-up starts the moment the Tensor queue
    # comes up -- the junk results land in a PSUM bank that the first real
    # matmul resets with start=True.  A 1-column scratch pulls the
    # Derivative_Erf ACT table load forward to overlap the input DMAs.
    warm = nc.alloc_sbuf_tensor("warm_junk", [128, 128], F16).ap()
    scratch = const_pool.tile([128, 1], F16)
    nc.gpsimd.memset(scratch, 0.0)
    # ---- ramp-critical transfers.  The sync HWDGE queue is the fastest
    # and most deterministic, so it carries, in order: the d-tile-0 x rows
    # (+params prefix, gating all basis production), then the first PE
    # block's weights.  The slower Activation HWDGE queue only gets the
    # non-critical second x d-tile (needed ~10us later). ----
    w_tile = w_pool.tile([128, K_SPLINE, OUT_DIM], F16)
    xas = [x_pool.tile([128, P_PRE + b_shard], F16, name=f"xa{dt}")
           for dt in range(N_DT)]
    nc.sync.dma_start(out=xas[0], in_=xaug[0:128, :])
    nc.scalar.dma_start(out=w_tile[:, 0:KB, :], in_=w_sp[:, 0:KB, :])
    nc.scalar.dma_start(out=xas[1], in_=xaug[128:256, :])
    nc.scalar.activation(out=scratch, in_=scratch,
                         func=mybir.ActivationFunctionType.Derivative_Erf)
    params_sb = xas[0][:, 0:P_PRE].bitcast(F32)          # [128, N_PAR]
    uscale_sb = params_sb[:, 0:N_DT]
    abias_sb = params_sb[:, N_DT:N_DT + N_DT * G]        # ushift - c_g
    bias_sb = params_sb[:, N_DT + N_DT * G:]
    xs = [xa[:, P_PRE:] for xa in xas]                   # [128, b_shard] views

    wb_tile = w_pool.tile([128, K_BASE, OUT_DIM], F16)

    # relu(x) for the base matmul, whole shard per d-tile (DVE); dt 2/3 are
    # emitted later, after their (gated) x DMAs, to keep reads after writes
    relus = []

    def emit_relu(dt):
        rl = x_pool.tile([128, b_shard], F16, name=f"rl{dt}")
        nc.vector.tensor_scalar_max(out=rl, in0=xs[dt], scalar1=0.0)
        relus.append(rl)

    def emit_basis(ch, dts=range(N_DT), basis=None):
        b0 = ch * b_chunk
        basis = [] if basis is None else basis
        for dt in dts:
            for g in range(G):
                bt = basis_pool.tile([128, b_chunk], F16, tag="basis")
                # basis_g = sqrt(pi)/2 * DErf(uscale*x + (ushift - c_g));
                # BN folded into ACT scale+bias, sqrt(pi)/2 into W_spline.
                # The very first tile is produced in halves: kc0's matmuls
                # consume the bh=0 slice first, so the PE can start on the
                # first 512 columns ~0.6us earlier.
                hs = 512 if (ch == 0 and dt == 0 and g == 0) else b_chunk
                for h in range(0, b_chunk, hs):
                    nc.scalar.activation(
                        out=bt[:, h:h + hs],
                        in_=xs[dt][:, b0 + h:b0 + h + hs],
                        func=mybir.ActivationFunctionType.Derivative_Erf,
                        scale=uscale_sb[:, dt:dt + 1],
                        bias=abias_sb[:, dt * G + g:dt * G + g + 1],
                    )
                basis.append(bt)
        return basis

    def operands(kc, osub, bh, b0, basis):
        if kc < K_SPLINE:
            return (w_tile[:, kc, osub * 128:(osub + 1) * 128],
                    basis[kc][:, bh * 512:(bh + 1) * 512])
        dt = kc - K_SPLINE
        return (wb_tile[:, dt, osub * 128:(osub + 1) * 128],
                relus[dt][:, b0 + bh * 512:b0 + (bh + 1) * 512])

    def emit_main_blocks(ch, psums, basis):
        b0 = ch * b_chunk
        # kc-outer: all 8 psum tiles consume basis[kc] (1.73us) before the
        # next K-chunk is touched, pacing the PE to ACT production (~1.4us)
        for kc in range(K_LAST):
            for bh in range(n_bh):
                for osub in range(N_OSUB):
                    lhsT, rhs = operands(kc, osub, bh, b0, basis)
                    nc.tensor.matmul(
                        psums[osub * n_bh + bh], lhsT=lhsT, rhs=rhs,
                        start=(kc == 0), stop=False)

    def emit_final_block(ch, psums, basis):
        b0 = ch * b_chunk
        last_ch = (ch == n_chunks - 1)
        for osub in range(N_OSUB):
            ot = out_pool.tile([128, b_chunk], F16, tag="ot")
            for bh in range(n_bh):
                ps = psums[osub * n_bh + bh]
                for kc in range(K_LAST, k_total):
                    lhsT, rhs = operands(kc, osub, bh, b0, basis)
                    nc.tensor.matmul(
                        ps, lhsT=lhsT, rhs=rhs,
                        start=False, stop=(kc == k_total - 1))
                # epilogue: ot = (psum + bias) + x_T  (residual + bias fused)
                nc.vector.scalar_tensor_tensor(
                    out=ot[:, bh * 512:(bh + 1) * 512], in0=ps,
                    scalar=bias_sb[:, osub:osub + 1],
                    in1=xs[osub][:, b0 + bh * 512:b0 + (bh + 1) * 512],
                    op0=mybir.AluOpType.add, op1=mybir.AluOpType.add)
            # stores: 2KB rows; the last chunk's stores alternate between
            # the sync and Activation HWDGE queues so the final drain (which
            # gates the NEFF teardown) proceeds on two queues in parallel,
            # and the very last store is split by partition halves too
            if last_ch and osub == N_OSUB - 1:
                # the very last store gates the NEFF teardown: split it by
                # partitions across the two HWDGE queues, 92/36 because the
                # Activation queue drains ~2.7x slower than sync (measured;
                # a 3-way split adding SWDGE was worse: software desc-gen
                # issues the last piece later)
                o0 = osub * 128
                nc.sync.dma_start(
                    out=out_t[o0:o0 + 92, b0:b0 + b_chunk], in_=ot[0:92, :])
                nc.scalar.dma_start(
                    out=out_t[o0 + 92:o0 + 128, b0:b0 + b_chunk],
                    in_=ot[92:128, :])
            else:
                # last chunk: only the early-issued osub 1 store rides the
                # slow Activation queue; the rest go on sync
                eng = nc.scalar if (last_ch and osub == 1) else nc.sync
                eng.dma_start(
                    out=out_t[osub * 128:(osub + 1) * 128, b0:b0 + b_chunk],
                    in_=ot)

    def alloc_psums(ch):
        return [psum_pool.tile([128, 512], F32, tag="ps", name=f"ps{ch}_{i}")
                for i in range(N_OSUB * n_bh)]

    psums0 = alloc_psums(0)
    # PE warm-up: dependency-free matmuls into psums0[0] release the HAM
    # clock throttle before the first real matmul arrives; the real kc==0
    # matmul has start=True, which resets the bank, so the junk results
    # never reach the output.
    for j in range(N_WARM):
        nc.tensor.matmul(psums0[0][:, 0:128], lhsT=warm, rhs=warm,
                         start=(j == 0), stop=(j == N_WARM - 1))

    emit_relu(0)
    emit_relu(1)
    basis = emit_basis(0, dts=range(0, 2))

    # ---- bulk transfers (remaining x d-tiles, remaining weights) are NOT
    # needed until ~23us in, but issuing them immediately makes all 8
    # cores' bulk streams contend with the ramp-critical xa0/w6 DMAs on
    # the shared HBM.  A dummy gpsimd op reading an early basis tile plus a
    # manual scheduler wait hold the SWDGE queue until the ramp transfers
    # have landed. ----
    gate_dummy = const_pool.tile([128, 1], F16)
    with tc.tile_wait_until(0.012):
        nc.gpsimd.tensor_scalar_max(out=gate_dummy, in0=basis[2][:, 0:1],
                                    scalar1=0.0)
        nc.gpsimd.dma_start(out=w_tile[:, KB:KB + W_SLICE, :],
                            in_=w_sp[:, KB:KB + W_SLICE, :])
        nc.gpsimd.dma_start(out=xas[2], in_=xaug[256:384, :])
        for ws in range(KB + W_SLICE, K_SPLINE, W_SLICE):
            we = min(ws + W_SLICE, K_SPLINE)
            nc.gpsimd.dma_start(out=w_tile[:, ws:we, :],
                                in_=w_sp[:, ws:we, :])
            if ws == KB + W_SLICE:
                nc.gpsimd.dma_start(out=xas[3], in_=xaug[384:512, :])
        nc.gpsimd.dma_start(out=wb_tile, in_=w_b)

    # readers of the gated xa2/xa3 tiles, emitted after their DMA writes
    emit_relu(2)
    emit_relu(3)
    basis = emit_basis(0, dts=range(2, N_DT), basis=basis)
    psums = psums0
    for ch in range(n_chunks):
        emit_main_blocks(ch, psums, basis)
        cur_basis, cur_psums = basis, psums
        if ch + 1 < n_chunks:
            basis = emit_basis(ch + 1)
            psums = alloc_psums(ch + 1)
        emit_final_block(ch, cur_psums, cur_basis)


def build_program(b_shard=B_SHARD, b_chunk=B_CHUNK):
    nc = bacc.Bacc("TRN2", target_bir_lowering=False, debug=False,
                   num_devices=N_CORES)
    xaug = nc.dram_tensor("xaug", [IN_DIM, P_PRE + b_shard], F16,
                          kind="ExternalInput").ap()
    w_sp = nc.dram_tensor("w_sp", [128, K_SPLINE, OUT_DIM], F16,
                          kind="ExternalInput").ap()
    w_b = nc.dram_tensor("w_base", [128, K_BASE, OUT_DIM], F16,
                         kind="ExternalInput").ap()
    out_t = nc.dram_tensor("out_t", [OUT_DIM, b_shard], F16,
                           kind="ExternalOutput").ap()
    with tile.TileContext(nc) as tc:
        _body(tc, xaug, w_sp, w_b, out_t, b_shard, b_chunk)
    nc.compile()
    return nc


def make_in_maps(x, gamma, beta, moving_mean, moving_var, W_spline, b_spline,
                 W_base, b_base, n_cores=N_CORES):
    """Host-side preprocessing + per-core input shards."""
    x = np.asarray(x, dtype=np.float32)
    gamma = np.asarray(gamma, dtype=np.float32)
    beta = np.asarray(beta, dtype=np.float32)
    moving_mean = np.asarray(moving_mean, dtype=np.float32)
    moving_var = np.asarray(moving_var, dtype=np.float32)
    W_spline = np.asarray(W_spline, dtype=np.float32)
    W_base = np.asarray(W_base, dtype=np.float32)
    b_spline = np.asarray(b_spline, dtype=np.float32)
    b_base = np.asarray(b_base, dtype=np.float32)

    scale = gamma / np.sqrt(moving_var + np.float32(BN_EPS))
    shift = beta - moving_mean * scale
    uscale = (scale / np.float32(DENOM)).astype(np.float32)
    ushift = (shift / np.float32(DENOM)).astype(np.float32)

    x16t = np.ascontiguousarray(x.T.astype(np.float16))  # [D, B]
    # K-order on chip is (dt, g, d_in): kc = dt*8+g covers d in
    # [dt*128, (dt+1)*128) at grid g.  W_spline rows are (d, g)-ordered.
    w_r = (W_spline.reshape(N_DT, 128, G, OUT_DIM)
           .transpose(0, 2, 1, 3)            # (dt, g, d_in, o)
           .reshape(K_SPLINE, 128, OUT_DIM)
           .transpose(1, 0, 2))              # (d_in, kc, o)
    w_sp = np.ascontiguousarray(w_r * np.float32(np.sqrt(np.pi) / 2.0)
                               ).astype(np.float16)
    w_b = np.ascontiguousarray(
        W_base.reshape(K_BASE, 128, OUT_DIM).transpose(1, 0, 2)
    ).astype(np.float16)
    bias_o = (b_spline + b_base).astype(np.float32)
    c = _grid_consts()
    # params layout: [uscale (dt)], [ushift - c_g (dt, g)], [out bias (osub)]
    params = np.empty((128, N_PAR), np.float32)
    params[:, 0:N_DT] = uscale.reshape(N_DT, 128).T
    ush = ushift.reshape(N_DT, 128)
    for dt in range(N_DT):
        for g in range(G):
            params[:, N_DT + dt * G + g] = ush[dt] - c[g]
    params[:, N_DT + N_DT * G:] = bias_o.reshape(N_OSUB, 128).T
    params16 = params.view(np.float16)       # [128, 2*N_PAR] bit pairs

    b_shard = x.shape[0] // n_cores
    in_maps = []
    for ci in range(n_cores):
        xaug = np.zeros((IN_DIM, P_PRE + b_shard), np.float16)
        xaug[0:128, 0:P_PRE] = params16
        xaug[:, P_PRE:] = x16t[:, ci * b_shard:(ci + 1) * b_shard]
        in_maps.append({
            "xaug": xaug,
            "w_sp": w_sp,
            "w_base": w_b,
        })
    return in_maps


_PROGRAM = None


def kernel(x, gamma, beta, moving_mean, moving_var, W_spline, b_spline,
           W_base, b_base):
    global _PROGRAM
    if _PROGRAM is None:
        _PROGRAM = build_program()
    in_maps = make_in_maps(x, gamma, beta, moving_mean, moving_var,
                           W_spline, b_spline, W_base, b_base)
    res = run_bass_kernel_spmd(_PROGRAM, in_maps, core_ids=list(range(N_CORES)))
    out = np.concatenate(
        [np.ascontiguousarray(res.results[ci]["out_t"].T)
         for ci in range(N_CORES)], axis=0)
    return out.astype(np.float32)


# revision 50
# speedup vs baseline: 1.0086x; 1.0086x over previous
"""Trainium2 Bass kernel for EnhancedFastKANLayer.

Reference computation (B=16384, D=O=512, G=8 grids):
    x_norm = (x - mean) * rsqrt(var + eps) * gamma + beta          # BN inference
    basis[b,d,g] = exp(-((x_norm[b,d] - grid[g]) / denom)^2)       # RBF expansion
    out = basis.reshape(B, D*G) @ W_spline + b_spline
        + relu(x) @ W_base + b_base + x

Strategy:
  - Data parallel: batch 16384 sharded 8 ways (2048 rows/core); weights
    replicated. No collectives.
  - All on-chip compute happens in the transposed layout [feature, batch]:
    the output is produced as out_T [O, B_shard] and transposed back on the
    host. This makes BN/basis per-partition-scalar ops and lets the spline
    matmul consume basis tiles directly as the moving operand.
  - RBF via ScalarE Derivative_Erf: d/dx erf(x) = 2/sqrt(pi)*exp(-x^2), so
    basis_g = sqrt(pi)/2 * Derivative_Erf(uscale*x + (ushift - c_g)) -- ONE
    ACT op per (d-tile, grid): BN is folded into the ACT per-partition
    scale+bias operands, and sqrt(pi)/2 into W_spline on the host.
  - DMA cost on this part is ~0.4us per partition-row packet per ring,
    nearly independent of row size, so the kernel minimizes DMA count and
    maximizes row length:
      * x is pre-transposed/pre-cast to fp16 [D, B_shard] and loaded in 4
        DMAs of [128, 2128] (4.25KB rows) covering BOTH batch chunks; the
        first 80 fp16 columns of the d-tile-0 rows carry all fp32 scalar
        params bit-cast into fp16 pairs (recovered on chip via bitcast).
      * the first spline-weight slice rides the otherwise-idle Activation
        HWDGE queue so it lands in parallel with x/params on the sync queue.
      * remaining weights stream on the gpsimd SWDGE queue in 7KB rows.
  - Matmul: out_T[o_sub, b] accumulates 36 matmuls per PSUM tile: 32 spline
    K-chunks + 4 relu(x)@W_base K-chunks.  The residual (+x) and the output
    bias are fused into the single epilogue DVE op per psum tile:
        ot = (psum + bias) + x_T        (scalar_tensor_tensor)
  - Matmul emission is kc-outer (all 8 PSUM tiles per K-chunk before the
    next K-chunk): the PE consumes one basis tile per 1.73us while ACT
    produces one per ~1.4us, so the pipeline runs stall-free from the first
    tile with no extra lookahead needed.
  - Output is fp16 (upcast on host), one DMA per 128-row osub across the
    whole 1024-col chunk (2KB rows); the last chunk's stores are split
    across the sync and Activation HWDGE queues to halve the final drain.
  - PE warm-up matmuls on an uninitialized SBUF region (results discarded
    by the first real matmul's start=True bank reset) release the HAM clock
    throttle (~3.4us of sustained PE activity at half clock) while the
    ramp-critical DMAs land; the warm-up count is tuned so the PE rolls
    straight from warm-up into the real stream with no idle gap (an idle
    PE re-arms the throttle).
"""

import numpy as np
from contextlib import ExitStack

import concourse.bass as bass
import concourse.tile as tile
from concourse import bacc, mybir
from concourse._compat import with_exitstack
from concourse.bass_utils import run_bass_kernel_spmd

N_CORES = 8
BATCH, IN_DIM, OUT_DIM, G = 16384, 512, 512, 8
B_SHARD = BATCH // N_CORES          # 2048
B_CHUNK = 1024                      # batch columns processed per chunk
GRID_MIN, GRID_MAX, BN_EPS = -2.0, 2.0, 1e-3
DENOM = (GRID_MAX - GRID_MIN) / G   # 0.5
N_DT = IN_DIM // 128                # 4 d-tiles
K_SPLINE = N_DT * G                 # 32 spline K-chunks
K_BASE = N_DT                       # 4 base K-chunks
N_OSUB = OUT_DIM // 128             # 4 output partition tiles
N_PAR = N_DT + N_DT * G + N_OSUB    # 40 fp32 scalar params
P_PRE = 2 * N_PAR                   # 80 fp16 columns carrying the params

F32 = mybir.dt.float32
F16 = mybir.dt.float16

N_WARM = 74                         # PE warm-up matmuls (HAM clock ramp)


def _grid_consts():
    grid = np.linspace(GRID_MIN, GRID_MAX, G, dtype=np.float32)
    c = (grid / np.float32(DENOM)).astype(np.float32)        # grid in u-units
    return c


@with_exitstack
def _body(ctx, tc, xaug, w_sp, w_b, out_t, b_shard, b_chunk):
    nc = tc.nc
    n_chunks = b_shard // b_chunk
    n_bh = b_chunk // 512            # 512-wide moving-operand slices
    k_total = K_SPLINE + K_BASE      # residual handled in the epilogue
    KB = 6                           # K-chunks per PE block
    K_LAST = 30                      # kc >= K_LAST run per-bank, fused w/ epilogue
    W_SLICE = 7                      # spline weight K-chunks per SWDGE DMA

    const_pool = ctx.enter_context(tc.tile_pool(name="const", bufs=1))
    w_pool = ctx.enter_context(tc.tile_pool(name="w", bufs=1))
    x_pool = ctx.enter_context(tc.tile_pool(name="x", bufs=1))
    # spline basis tiles: 32 per chunk stay resident through the chunk's
    # matmul phase; extra slots let the next chunk's production run ahead.
    basis_pool = ctx.enter_context(tc.tile_pool(name="basis", bufs=K_SPLINE + 12))
    psum_pool = ctx.enter_context(
        tc.tile_pool(name="psum", bufs=8, space="PSUM"))
    out_pool = ctx.enter_context(tc.tile_pool(name="outs", bufs=6))

    # ---- warm-up operand: a raw (untracked) SBUF region read
    # uninitialized, so the PE warm-up starts the moment the Tensor queue
    # comes up -- the junk results land in a PSUM bank that the first real
    # matmul resets with start=True.  A 1-column scratch pulls the
    # Derivative_Erf ACT table load forward to overlap the input DMAs.
    warm = nc.alloc_sbuf_tensor("warm_junk", [128, 128], F16).ap()
    scratch = const_pool.tile([128, 1], F16)
    nc.gpsimd.memset(scratch, 0.0)
    # ---- ramp-critical transfers.  The sync HWDGE queue is the fastest
    # and most deterministic, so it carries, in order: the d-tile-0 x rows
    # (+params prefix, gating all basis production), then the first PE
    # block's weights.  The slower Activation HWDGE queue only gets the
    # non-critical second x d-tile (needed ~10us later). ----
    w_tile = w_pool.tile([128, K_SPLINE, OUT_DIM], F16)
    xas = [x_pool.tile([128, P_PRE + b_shard], F16, name=f"xa{dt}")
           for dt in range(N_DT)]
    nc.sync.dma_start(out=xas[0], in_=xaug[0:128, :])
    nc.scalar.dma_start(out=w_tile[:, 0:KB, :], in_=w_sp[:, 0:KB, :])
    nc.scalar.dma_start(out=xas[1], in_=xaug[128:256, :])
    nc.scalar.activation(out=scratch, in_=scratch,
                         func=mybir.ActivationFunctionType.Derivative_Erf)
    params_sb = xas[0][:, 0:P_PRE].bitcast(F32)          # [128, N_PAR]
    uscale_sb = params_sb[:, 0:N_DT]
    abias_sb = params_sb[:, N_DT:N_DT + N_DT * G]        # ushift - c_g
    bias_sb = params_sb[:, N_DT + N_DT * G:]
    xs = [xa[:, P_PRE:] for xa in xas]                   # [128, b_shard] views

    wb_tile = w_pool.tile([128, K_BASE, OUT_DIM], F16)

    # relu(x) for the base matmul, whole shard per d-tile (DVE); dt 2/3 are
    # emitted later, after their (gated) x DMAs, to keep reads after writes
    relus = []

    def emit_relu(dt):
        rl = x_pool.tile([128, b_shard], F16, name=f"rl{dt}")
        nc.vector.tensor_scalar_max(out=rl, in0=xs[dt], scalar1=0.0)
        relus.append(rl)

    def emit_basis(ch, dts=range(N_DT), basis=None):
        b0 = ch * b_chunk
        basis = [] if basis is None else basis
        for dt in dts:
            for g in range(G):
                bt = basis_pool.tile([128, b_chunk], F16, tag="basis")
                # basis_g = sqrt(pi)/2 * DErf(uscale*x + (ushift - c_g));
                # BN folded into ACT scale+bias, sqrt(pi)/2 into W_spline.
                # The very first tile is produced in halves: kc0's matmuls
                # consume the bh=0 slice first, so the PE can start on the
                # first 512 columns ~0.6us earlier.
                hs = 512 if (ch == 0 and dt == 0 and g == 0) else b_chunk
                for h in range(0, b_chunk, hs):
                    nc.scalar.activation(
                        out=bt[:, h:h + hs],
                        in_=xs[dt][:, b0 + h:b0 + h + hs],
                        func=mybir.ActivationFunctionType.Derivative_Erf,
                        scale=uscale_sb[:, dt:dt + 1],
                        bias=abias_sb[:, dt * G + g:dt * G + g + 1],
                    )
                basis.append(bt)
        return basis

    def operands(kc, osub, bh, b0, basis):
        if kc < K_SPLINE:
            return (w_tile[:, kc, osub * 128:(osub + 1) * 128],
                    basis[kc][:, bh * 512:(bh + 1) * 512])
        dt = kc - K_SPLINE
        return (wb_tile[:, dt, osub * 128:(osub + 1) * 128],
                relus[dt][:, b0 + bh * 512:b0 + (bh + 1) * 512])

    def emit_main_blocks(ch, psums, basis):
        b0 = ch * b_chunk
        # kc-outer: all 8 psum tiles consume basis[kc] (1.73us) before the
        # next K-chunk is touched, pacing the PE to ACT production (~1.4us)
        for kc in range(K_LAST):
            for bh in range(n_bh):
                for osub in range(N_OSUB):
                    lhsT, rhs = operands(kc, osub, bh, b0, basis)
                    nc.tensor.matmul(
                        psums[osub * n_bh + bh], lhsT=lhsT, rhs=rhs,
                        start=(kc == 0), stop=False)

    def emit_final_block(ch, psums, basis):
        b0 = ch * b_chunk
        last_ch = (ch == n_chunks - 1)
        for osub in range(N_OSUB):
            ot = out_pool.tile([128, b_chunk], F16, tag="ot")
            for bh in range(n_bh):
                ps = psums[osub * n_bh + bh]
                for kc in range(K_LAST, k_total):
                    lhsT, rhs = operands(kc, osub, bh, b0, basis)
                    nc.tensor.matmul(
                        ps, lhsT=lhsT, rhs=rhs,
                        start=False, stop=(kc == k_total - 1))
                # epilogue: ot = (psum + bias) + x_T  (residual + bias fused)
                nc.vector.scalar_tensor_tensor(
                    out=ot[:, bh * 512:(bh + 1) * 512], in0=ps,
                    scalar=bias_sb[:, osub:osub + 1],
                    in1=xs[osub][:, b0 + bh * 512:b0 + (bh + 1) * 512],
                    op0=mybir.AluOpType.add, op1=mybir.AluOpType.add)
            # stores: 2KB rows; the last chunk's stores alternate between
            # the sync and Activation HWDGE queues so the final drain (which
            # gates the NEFF teardown) proceeds on two queues in parallel,
            # and the very last store is split by partition halves too
            if last_ch and osub == N_OSUB - 1:
                # the very last store gates the NEFF teardown: split it by
                # partition halves across the two HWDGE queues (measured
                # best; 92/36 rebalancing and 3-way SWDGE splits both
                # made the final drain longer)
                o0 = osub * 128
                nc.sync.dma_start(
                    out=out_t[o0:o0 + 64, b0:b0 + b_chunk], in_=ot[0:64, :])
                nc.scalar.dma_start(
                    out=out_t[o0 + 64:o0 + 128, b0:b0 + b_chunk],
                    in_=ot[64:128, :])
            else:
                eng = nc.scalar if (last_ch and osub % 2 == 0) else nc.sync
                eng.dma_start(
                    out=out_t[osub * 128:(osub + 1) * 128, b0:b0 + b_chunk],
                    in_=ot)

    def alloc_psums(ch):
        return [psum_pool.tile([128, 512], F32, tag="ps", name=f"ps{ch}_{i}")
                for i in range(N_OSUB * n_bh)]

    psums0 = alloc_psums(0)
    # PE warm-up: dependency-free matmuls into psums0[0] release the HAM
    # clock throttle before the first real matmul arrives; the real kc==0
    # matmul has start=True, which resets the bank, so the junk results
    # never reach the output.
    for j in range(N_WARM):
        nc.tensor.matmul(psums0[0][:, 0:128], lhsT=warm, rhs=warm,
                         start=(j == 0), stop=(j == N_WARM - 1))

    emit_relu(0)
    emit_relu(1)
    basis = emit_basis(0, dts=range(0, 2))

    # ---- bulk transfers (remaining x d-tiles, remaining weights) are NOT
    # needed until ~23us in, but issuing them immediately makes all 8
    # cores' bulk streams contend with the ramp-critical xa0/w6 DMAs on
    # the shared HBM.  A dummy gpsimd op reading an early basis tile plus a
    # manual scheduler wait hold the SWDGE queue until the ramp transfers
    # have landed. ----
    gate_dummy = const_pool.tile([128, 1], F16)
    with tc.tile_wait_until(0.012):
        nc.gpsimd.tensor_scalar_max(out=gate_dummy, in0=basis[2][:, 0:1],
                                    scalar1=0.0)
        nc.gpsimd.dma_start(out=w_tile[:, KB:KB + W_SLICE, :],
                            in_=w_sp[:, KB:KB + W_SLICE, :])
        nc.gpsimd.dma_start(out=xas[2], in_=xaug[256:384, :])
        for ws in range(KB + W_SLICE, K_SPLINE, W_SLICE):
            we = min(ws + W_SLICE, K_SPLINE)
            nc.gpsimd.dma_start(out=w_tile[:, ws:we, :],
                                in_=w_sp[:, ws:we, :])
            if ws == KB + W_SLICE:
                nc.gpsimd.dma_start(out=xas[3], in_=xaug[384:512, :])
        nc.gpsimd.dma_start(out=wb_tile, in_=w_b)

    # readers of the gated xa2/xa3 tiles, emitted after their DMA writes
    emit_relu(2)
    emit_relu(3)
    basis = emit_basis(0, dts=range(2, N_DT), basis=basis)
    psums = psums0
    for ch in range(n_chunks):
        emit_main_blocks(ch, psums, basis)
        cur_basis, cur_psums = basis, psums
        if ch + 1 < n_chunks:
            basis = emit_basis(ch + 1)
            psums = alloc_psums(ch + 1)
        emit_final_block(ch, cur_psums, cur_basis)


def build_program(b_shard=B_SHARD, b_chunk=B_CHUNK):
    nc = bacc.Bacc("TRN2", target_bir_lowering=False, debug=False,
                   num_devices=N_CORES)
    xaug = nc.dram_tensor("xaug", [IN_DIM, P_PRE + b_shard], F16,
                          kind="ExternalInput").ap()
    w_sp = nc.dram_tensor("w_sp", [128, K_SPLINE, OUT_DIM], F16,
                          kind="ExternalInput").ap()
    w_b = nc.dram_tensor("w_base", [128, K_BASE, OUT_DIM], F16,
                         kind="ExternalInput").ap()
    out_t = nc.dram_tensor("out_t", [OUT_DIM, b_shard], F16,
                           kind="ExternalOutput").ap()
    with tile.TileContext(nc) as tc:
        _body(tc, xaug, w_sp, w_b, out_t, b_shard, b_chunk)
    nc.compile()
    return nc


def make_in_maps(x, gamma, beta, moving_mean, moving_var, W_spline, b_spline,
                 W_base, b_base, n_cores=N_CORES):
    """Host-side preprocessing + per-core input shards."""
    x = np.asarray(x, dtype=np.float32)
    gamma = np.asarray(gamma, dtype=np.float32)
    beta = np.asarray(beta, dtype=np.float32)
    moving_mean = np.asarray(moving_mean, dtype=np.float32)
    moving_var = np.asarray(moving_var, dtype=np.float32)
    W_spline = np.asarray(W_spline, dtype=np.float32)
    W_base = np.asarray(W_base, dtype=np.float32)
    b_spline = np.asarray(b_spline, dtype=np.float32)
    b_base = np.asarray(b_base, dtype=np.float32)

    scale = gamma / np.sqrt(moving_var + np.float32(BN_EPS))
    shift = beta - moving_mean * scale
    uscale = (scale / np.float32(DENOM)).astype(np.float32)
    ushift = (shift / np.float32(DENOM)).astype(np.float32)

    x16t = np.ascontiguousarray(x.T.astype(np.float16))  # [D, B]
    # K-order on chip is (dt, g, d_in): kc = dt*8+g covers d in
    # [dt*128, (dt+1)*128) at grid g.  W_spline rows are (d, g)-ordered.
    w_r = (W_spline.reshape(N_DT, 128, G, OUT_DIM)
           .transpose(0, 2, 1, 3)            # (dt, g, d_in, o)
           .reshape(K_SPLINE, 128, OUT_DIM)
           .transpose(1, 0, 2))              # (d_in, kc, o)
    w_sp = np.ascontiguousarray(w_r * np.float32(np.sqrt(np.pi) / 2.0)
                               ).astype(np.float16)
    w_b = np.ascontiguousarray(
        W_base.reshape(K_BASE, 128, OUT_DIM).transpose(1, 0, 2)
    ).astype(np.float16)
    bias_o = (b_spline + b_base).astype(np.float32)
    c = _grid_consts()
    # params layout: [uscale (dt)], [ushift - c_g (dt, g)], [out bias (osub)]
    params = np.empty((128, N_PAR), np.float32)
    params[:, 0:N_DT] = uscale.reshape(N_DT, 128).T
    ush = ushift.reshape(N_DT, 128)
    for dt in range(N_DT):
        for g in range(G):
            params[:, N_DT + dt * G + g] = ush[dt] - c[g]
    params[:, N_DT + N_DT * G:] = bias_o.reshape(N_OSUB, 128).T
    params16 = params.view(np.float16)       # [128, 2*N_PAR] bit pairs

    b_shard = x.shape[0] // n_cores
    in_maps = []
    for ci in range(n_cores):
        xaug = np.zeros((IN_DIM, P_PRE + b_shard), np.float16)
        xaug[0:128, 0:P_PRE] = params16
        xaug[:, P_PRE:] = x16t[:, ci * b_shard:(ci + 1) * b_shard]
        in_maps.append({
            "xaug": xaug,
            "w_sp": w_sp,
            "w_base": w_b,
        })
    return in_maps


_PROGRAM = None


def kernel(x, gamma, beta, moving_mean, moving_var, W_spline, b_spline,
           W_base, b_base):
    global _PROGRAM
    if _PROGRAM is None:
        _PROGRAM = build_program()
    in_maps = make_in_maps(x, gamma, beta, moving_mean, moving_var,
                           W_spline, b_spline, W_base, b_base)
    res = run_bass_kernel_spmd(_PROGRAM, in_maps, core_ids=list(range(N_CORES)))
    out = np.concatenate(
        [np.ascontiguousarray(res.results[ci]["out_t"].T)
         for ci in range(N_CORES)], axis=0)
    return out.astype(np.float32)
